# revision 13
# baseline (speedup 1.0000x reference)
"""Trainium2 Bass kernel for nn_Attention_78048145703090 (sparse_attention).

Math: the reference's [N,N] attention is rank-1 structured. Every row n of the
logit matrix is a_n * t where t[m] = q_center . k_m is one shared score vector
per sample and a_n = scale * exp(1 - dist_n) depends only on the grid distance
of n from the center. Softmax rows therefore only depend on a_n, and the row
output out(a) = softmax(a*t) @ V is a smooth function of the scalar a. The
kernel evaluates D=32 uniformly spaced knots in a and expands to the 4096 rows
with a piecewise-linear interpolation matmul (interp error ~4e-5, far below
the bf16 noise floor).

Per core (one sample), m in chunks of 128 rows, pipelined in quarters:
  t    = x @ u            u = wk^T q_c folded on the host (O(C^2) prep);
                          DVE multiply (bf16, 2x mode) + reduce (f32 accum)
  L    = outer(t, a_j)    GpSimd broadcast-multiply (f32)
  E    = exp(L)           one wide Act op per quarter, bf16 out
  ytd  = [x|1]^T E        32 accumulating bf16 matmuls -> [65, 32] f32 PSUM
                          (row 64 = den, via the ones column of x1b)
  g    = proj(ytd/den)    tiny [32, 64] bf16 chain, divide via Act scale
  out  = T^T g            32 bf16 matmuls [32j,128n]^T x [32j,64c] land the
                          output in natural [n, c] layout; no transposes
x is shipped from the host already in bf16 (halves the input DMA); all PE
contractions are bf16 (fp32 matmuls cost 2 half-rate passes + double
LDWEIGHTS on TRN2), accumulation stays f32 in PSUM. Measured end-to-end
error ~4e-3 absmax-relative vs the f32 reference.

Sharding: data-parallel over B=8 across the 8 cores (one sample per core);
each core holds the full (tiny) weights.
"""

import sys

sys.path.insert(0, "/opt/trn_rl_repo")

import numpy as np

import concourse.bacc as bacc
import concourse.mybir as mybir
import concourse.tile as tile
from concourse import masks


def _install_profile_hook():
    """This image's antenv lacks axon_hooks; reconstruct it so
    run_bass_kernel_spmd(trace=True) can capture NTFF profiles. No-op for
    normal (untraced) runs."""
    import types

    try:
        import antenv.axon_hooks  # noqa: F401

        return
    except ImportError:
        pass
    try:
        import antenv

        m = types.ModuleType("antenv.axon_hooks")
        state = {"hook": None}
        m.set_axon_ntff_profile_hook = lambda h: state.__setitem__("hook", h)
        m.get_axon_ntff_profile_hook = lambda: state["hook"]
        sys.modules["antenv.axon_hooks"] = m
        antenv.axon_hooks = m
        from trn_agent_boot.trn_boot import _ntff_profile_via_ctypes

        m.set_axon_ntff_profile_hook(
            _ntff_profile_via_ctypes("/opt/axon/libaxon_pjrt.so")
        )
    except Exception:
        pass


_install_profile_hook()

from concourse.bass_utils import run_bass_kernel_spmd

B, H, W, C = 8, 64, 64, 64
N = H * W  # 4096
P = 128
NCH = N // P  # 32 chunks of 128 rows; chunk s holds rows {p*NCH+s}
CENTER = (H // 2) * W + (W // 2)  # 2080
SCALE = float(C) ** -0.5
F32 = mybir.dt.float32
BF16 = mybir.dt.bfloat16
D = 32  # interpolation knots in the temperature axis
NQ = 4  # pipeline quarters (8 chunks each)
QC = NCH // NQ  # 8
DG = 8  # output chunks per PSUM tile in the gather phase
NDG = NCH // DG  # 8

# ---- compile-time constants from the distance grid ----
import ml_dtypes

_yy, _xx = np.mgrid[0:H, 0:W]
_dist = np.sqrt(((_yy - H // 2) ** 2 + (_xx - W // 2) ** 2).astype(np.float32))
_a_n = (np.exp(np.float32(1.0) - _dist.reshape(-1)) * np.float32(SCALE)).astype(
    np.float32
)
AMAX = float(_a_n.max())
KH = AMAX / (D - 1)
A_KNOTS = (np.arange(D) * KH).astype(np.float32)  # [D]
_j = np.minimum((_a_n / KH).astype(np.int64), D - 2)
_frac = (_a_n / KH - _j).astype(np.float32)
_T = np.zeros((N, D), np.float32)
_T[np.arange(N), _j] += 1.0 - _frac
_T[np.arange(N), _j + 1] += _frac
# Tt[j, s*128 + p] = T[p*32 + s, j]: stationary strips per output chunk s
TT = np.ascontiguousarray(
    _T.reshape(P, NCH, D).transpose(2, 1, 0).reshape(D, N)
).astype(ml_dtypes.bfloat16)


def build_nc():
    nc = bacc.Bacc("TRN2", target_bir_lowering=False, debug=False, num_devices=B)
    # x pre-cast to bf16 on the host, one [128, 8, 64] quarter view per DMA
    xb = nc.dram_tensor("xb", [N, C], BF16, kind="ExternalInput")
    # wpk: [:,0:64]=[wv.T;bv], [:,64:128]=[wp.T;bp]  (bf16)
    wpk = nc.dram_tensor("wpk", [C + 1, 3 * C], BF16, kind="ExternalInput")
    auxb = nc.dram_tensor("auxb", [P, C], BF16, kind="ExternalInput")  # u bcast
    auxf = nc.dram_tensor("auxf", [P, D], F32, kind="ExternalInput")  # knots
    tt = nc.dram_tensor("tt", [D, N], BF16, kind="ExternalInput")
    out = nc.dram_tensor("out", [N, C], BF16, kind="ExternalOutput")

    xv = xb.ap().rearrange("(p i) c -> p i c", p=P)  # [128, 32, 64]
    ov = out.ap().rearrange("(p s) c -> p s c", p=P)

    with tile.TileContext(nc) as tc:
        with (
            tc.tile_pool(name="consts", bufs=1) as consts,
            tc.tile_pool(name="sb", bufs=1) as sb,
            tc.tile_pool(name="ps_yt", bufs=1, space="PSUM") as ps_yt,
            tc.tile_pool(name="ps_small", bufs=2, space="PSUM") as ps_small,
            tc.tile_pool(name="ps_g", bufs=4, space="PSUM") as ps_g,
        ):
            x1b = sb.tile([P, NCH, C + 1], BF16)
            o_big = sb.tile([P, NCH, C], BF16)

            # two independent DMA rings (sync=Q1, gpsimd=Q0); tiny operand
            # DMAs go first on each ring so they are not stuck behind the
            # 256KB x halves, and each ring carries one x half
            HH = NCH // 2
            auxb_sb = consts.tile([P, C], BF16)
            nc.gpsimd.dma_start(out=auxb_sb[:], in_=auxb[:])
            auxf_sb = consts.tile([P, D], F32)
            nc.gpsimd.dma_start(out=auxf_sb[:], in_=auxf[:])
            wpk_sb = consts.tile([C + 1, 3 * C], BF16)
            nc.gpsimd.dma_start(out=wpk_sb[:], in_=wpk[:])
            nc.sync.dma_start(out=x1b[:, 0:HH, 0:C], in_=xv[:, 0:HH, :])
            nc.sync.dma_start(out=x1b[:, HH:NCH, 0:C], in_=xv[:, HH:NCH, :])
            tt_sb = consts.tile([D, N], BF16)
            nc.gpsimd.dma_start(out=tt_sb[:], in_=tt[:])

            oneb65 = consts.tile([C + 1, 1], BF16)
            nc.vector.memset(oneb65[:], 1.0)
            dummy = consts.tile([1, 1], F32)
            nc.vector.memset(dummy[:], 0.0)
            dummy_o = consts.tile([1, 1], F32)
            # force the Exp act-table load at t~0 instead of mid-kernel
            nc.scalar.activation(
                out=dummy_o[:], in_=dummy[:], func=mybir.ActivationFunctionType.Exp
            )
            nc.vector.memset(x1b[:, :, C : C + 1], 1.0)  # den ones column

            wv1 = wpk_sb[:, 0:C]
            wpT = wpk_sb[0:C, C : 2 * C]
            bp_row = wpk_sb[C : C + 1, 2 * C : 3 * C]

            # -------- phase A: t, L=outer(t,a), E=exp(L), ytd=[x|1]^T E
            s_cols = sb.tile([P, NCH], F32)
            xu = sb.tile([P, NCH, C], BF16)
            lmat = sb.tile([P, NCH, D], F32)
            e_all = sb.tile([P, NCH, D], BF16)
            ytd_ps = ps_yt.tile([C + 1, D], F32)

            def bcast(ap, insert_at, size):
                lst = list(ap.ap)
                lst.insert(insert_at, [0, size])
                return type(ap)(tensor=ap.tensor, offset=ap.offset, ap=lst)

            ubc_bc = bcast(auxb_sb[:], 1, QC)  # [128, (8bc), 64]
            ab_bc = bcast(auxf_sb[:], 1, QC)  # [128, (8bc), 32]

            xr = sb.tile([P, NCH, 8], BF16)
            ubc_bc2 = bcast(auxb_sb[:], 1, HH)  # [128, (16bc), 64]
            xu4 = xu[:].rearrange("p i (a b) -> p i a b", a=8)
            xr_ap = xr[:]
            for h in range(2):
                hsl = slice(h * HH, (h + 1) * HH)
                # t[m] = x[m,:] . u : bf16 multiply (2x DVE mode), then a
                # 2-stage reduce (bf16 8-wide partials keep the 2x mode)
                nc.vector.tensor_mul(xu[:, hsl, :], x1b[:, hsl, 0:C], ubc_bc2)
                with nc.allow_low_precision(reason="8-wide bf16 partials"):
                    nc.vector.reduce_sum(
                        out=xr[:, hsl, :],
                        in_=xu4[:, hsl, :, :],
                        axis=mybir.AxisListType.X,
                    )
                nc.vector.reduce_sum(
                    out=s_cols[:, hsl],
                    in_=xr[:, hsl, :],
                    axis=mybir.AxisListType.X,
                )
            for q in range(NQ):
                sl = slice(q * QC, (q + 1) * QC)
                # L[:, i, j] = t[:, i] * a[j]
                s_bc = bcast(s_cols[:, sl], 2, D)  # [128, 8, (32bc)]
                nc.gpsimd.tensor_mul(lmat[:, sl, :], s_bc, ab_bc)
                nc.scalar.activation(
                    out=e_all[:, sl, :],
                    in_=lmat[:, sl, :],
                    func=mybir.ActivationFunctionType.Exp,
                )
                for i in range(q * QC, (q + 1) * QC):
                    nc.tensor.matmul(
                        ytd_ps[:],
                        x1b[:, i, :],
                        e_all[:, i, :],
                        start=(i == 0),
                        stop=(i == NCH - 1),
                    )

            # -------- phase B: knot outputs
            # g = (wp-proj(wv-proj(ytd)) + den (x) bp) / den; the divide by
            # den folds into the final copy (Act scale), bp folds via the
            # den (x) bp accumulate, so no transposes and no explicit o
            ytd_sb = sb.tile([C + 1, D], BF16)
            nc.vector.tensor_copy(out=ytd_sb[:], in_=ytd_ps[:])
            denc_ps = ps_small.tile([D, 1], F32, tag="m")
            nc.tensor.matmul(
                denc_ps[:],
                ytd_sb[C : C + 1, :],
                oneb65[C : C + 1, :],
                start=True,
                stop=True,
            )
            rc_sb = sb.tile([D, 1], F32)
            nc.vector.reciprocal(out=rc_sb[:], in_=denc_ps[:])
            numT_ps = ps_small.tile([C, D], F32, tag="m")
            nc.tensor.matmul(numT_ps[:], wv1, ytd_sb[:], start=True, stop=True)
            numT_sb = sb.tile([C, D], BF16)
            nc.vector.tensor_copy(out=numT_sb[:], in_=numT_ps[:])
            g_ps = ps_small.tile([D, C], F32, tag="m")
            nc.tensor.matmul(g_ps[:], numT_sb[:], wpT, start=True, stop=False)
            nc.tensor.matmul(
                g_ps[:], ytd_sb[C : C + 1, :], bp_row, start=False, stop=True
            )
            g_sb = sb.tile([D, C], BF16)
            nc.scalar.activation(
                out=g_sb[:],
                in_=g_ps[:],
                func=mybir.ActivationFunctionType.Copy,
                scale=rc_sb[:],
            )

            # -------- phase C: expand knots to 4096 rows, natural layout
            for gidx in range(NDG):
                obp = ps_g.tile([P, DG * C], F32, tag="g")
                for k in range(DG):
                    s = gidx * DG + k
                    nc.tensor.matmul(
                        obp[:, k * C : (k + 1) * C],
                        tt_sb[:, s * P : (s + 1) * P],
                        g_sb[:],
                        start=True,
                        stop=True,
                    )
                dst = o_big[:, gidx * DG : (gidx + 1) * DG, :]
                if gidx % 2 == 0:
                    nc.vector.tensor_copy(out=dst, in_=obp[:])
                else:
                    nc.scalar.copy(out=dst, in_=obp[:])
                s0 = gidx * DG
                eng = nc.sync if gidx % 2 == 0 else nc.gpsimd
                eng.dma_start(
                    out=ov[:, s0 : s0 + DG, :],
                    in_=o_big[:, s0 : s0 + DG, :],
                )

    nc.compile()
    return nc


_nc_cache = None


def _get_nc():
    global _nc_cache
    if _nc_cache is None:
        _nc_cache = build_nc()
    return _nc_cache


def make_in_maps(x, wq, bq, wk, bk, wv, bv, wp, bp):
    f = lambda a: np.ascontiguousarray(np.asarray(a, dtype=np.float32))
    x = f(x).reshape(B, N, C)
    wq, bq, wk = f(wq), f(bq), f(wk)
    wpk = np.zeros((C + 1, 3 * C), np.float32)
    wpk[0:C, 0:C] = f(wv).T
    wpk[C, 0:C] = f(bv)
    wpk[0:C, C : 2 * C] = f(wp).T
    wpk[C, 2 * C : 3 * C] = f(bp)
    wpk = np.ascontiguousarray(wpk.astype(ml_dtypes.bfloat16))
    tt = np.ascontiguousarray(TT)
    auxf = np.ascontiguousarray(np.broadcast_to(A_KNOTS[None, :], (P, D))).astype(
        np.float32
    )
    in_maps = []
    for b in range(B):
        u = ((x[b, CENTER] @ wq.T + bq) @ wk).astype(np.float32)  # [64]
        auxb = np.ascontiguousarray(
            np.broadcast_to(u[None, :], (P, C)).astype(ml_dtypes.bfloat16)
        )
        in_maps.append(
            {
                "xb": np.ascontiguousarray(x[b].astype(ml_dtypes.bfloat16)),
                "wpk": wpk,
                "auxb": auxb,
                "auxf": auxf,
                "tt": tt,
            }
        )
    return in_maps


def kernel_with_results(trace=False, **inputs):
    in_maps = make_in_maps(**inputs)
    nc = _get_nc()
    res = run_bass_kernel_spmd(nc, in_maps, core_ids=list(range(B)), trace=trace)
    out = np.stack(
        [np.asarray(r["out"]).astype(np.float32) for r in res.results], 0
    ).reshape(B, H, W, C)
    return out, res


def kernel(**inputs):
    out, _ = kernel_with_results(**inputs)
    return out


# revision 14
# speedup vs baseline: 1.0573x; 1.0573x over previous
"""Trainium2 Bass kernel for nn_Attention_78048145703090 (sparse_attention).

Math: the reference's [N,N] attention is rank-1 structured. Every row n of the
logit matrix is a_n * t where t[m] = q_center . k_m is one shared score vector
per sample and a_n = scale * exp(1 - dist_n) depends only on the grid distance
of n from the center. Softmax rows therefore only depend on a_n, and the row
output out(a) = softmax(a*t) @ V is a smooth function of the scalar a. The
kernel evaluates D=32 uniformly spaced knots in a and expands to the 4096 rows
with a piecewise-linear interpolation matmul (interp error ~4e-5, far below
the bf16 noise floor).

Per core (one sample), m in chunks of 128 rows, pipelined in quarters:
  t    = x @ u            u = wk^T q_c folded on the host (O(C^2) prep);
                          DVE multiply (bf16, 2x mode) + reduce (f32 accum)
  L    = outer(t, a_j)    GpSimd broadcast-multiply (f32)
  E    = exp(L)           one wide Act op per quarter, bf16 out
  ytd  = [x|1]^T E        32 accumulating bf16 matmuls -> [65, 32] f32 PSUM
                          (row 64 = den, via the ones column of x1b)
  g    = proj(ytd/den)    tiny [32, 64] bf16 chain, divide via Act scale
  out  = T^T g            32 bf16 matmuls [32j,128n]^T x [32j,64c] land the
                          output in natural [n, c] layout; no transposes
x is shipped from the host already in bf16 (halves the input DMA); all PE
contractions are bf16 (fp32 matmuls cost 2 half-rate passes + double
LDWEIGHTS on TRN2), accumulation stays f32 in PSUM. Measured end-to-end
error ~4e-3 absmax-relative vs the f32 reference.

Sharding: data-parallel over B=8 across the 8 cores (one sample per core);
each core holds the full (tiny) weights.
"""

import sys

sys.path.insert(0, "/opt/trn_rl_repo")

import numpy as np

import concourse.bacc as bacc
import concourse.mybir as mybir
import concourse.tile as tile
from concourse import masks


def _install_profile_hook():
    """This image's antenv lacks axon_hooks; reconstruct it so
    run_bass_kernel_spmd(trace=True) can capture NTFF profiles. No-op for
    normal (untraced) runs."""
    import types

    try:
        import antenv.axon_hooks  # noqa: F401

        return
    except ImportError:
        pass
    try:
        import antenv

        m = types.ModuleType("antenv.axon_hooks")
        state = {"hook": None}
        m.set_axon_ntff_profile_hook = lambda h: state.__setitem__("hook", h)
        m.get_axon_ntff_profile_hook = lambda: state["hook"]
        sys.modules["antenv.axon_hooks"] = m
        antenv.axon_hooks = m
        from trn_agent_boot.trn_boot import _ntff_profile_via_ctypes

        m.set_axon_ntff_profile_hook(
            _ntff_profile_via_ctypes("/opt/axon/libaxon_pjrt.so")
        )
    except Exception:
        pass


_install_profile_hook()

from concourse.bass_utils import run_bass_kernel_spmd

B, H, W, C = 8, 64, 64, 64
N = H * W  # 4096
P = 128
NCH = N // P  # 32 chunks of 128 rows; chunk s holds rows {p*NCH+s}
CENTER = (H // 2) * W + (W // 2)  # 2080
SCALE = float(C) ** -0.5
F32 = mybir.dt.float32
BF16 = mybir.dt.bfloat16
D = 32  # interpolation knots in the temperature axis
NQ = 4  # pipeline quarters (8 chunks each)
QC = NCH // NQ  # 8
DG = 8  # output chunks per PSUM tile in the gather phase
NDG = NCH // DG  # 8

# ---- compile-time constants from the distance grid ----
import ml_dtypes

_yy, _xx = np.mgrid[0:H, 0:W]
_dist = np.sqrt(((_yy - H // 2) ** 2 + (_xx - W // 2) ** 2).astype(np.float32))
_a_n = (np.exp(np.float32(1.0) - _dist.reshape(-1)) * np.float32(SCALE)).astype(
    np.float32
)
AMAX = float(_a_n.max())
KH = AMAX / (D - 1)
A_KNOTS = (np.arange(D) * KH).astype(np.float32)  # [D]
_j = np.minimum((_a_n / KH).astype(np.int64), D - 2)
_frac = (_a_n / KH - _j).astype(np.float32)
_T = np.zeros((N, D), np.float32)
_T[np.arange(N), _j] += 1.0 - _frac
_T[np.arange(N), _j + 1] += _frac
# Tt[j, s*128 + p] = T[p*32 + s, j]: stationary strips per output chunk s
TT = np.ascontiguousarray(
    _T.reshape(P, NCH, D).transpose(2, 1, 0).reshape(D, N)
).astype(ml_dtypes.bfloat16)


def build_nc():
    nc = bacc.Bacc("TRN2", target_bir_lowering=False, debug=False, num_devices=B)
    # x pre-cast to bf16 on the host, one [128, 8, 64] quarter view per DMA
    xb = nc.dram_tensor("xb", [N, C], BF16, kind="ExternalInput")
    # wpk: [:,0:64]=[wv.T;bv], [:,64:128]=[wp.T;bp]  (bf16)
    wpk = nc.dram_tensor("wpk", [C + 1, 3 * C], BF16, kind="ExternalInput")
    auxb = nc.dram_tensor("auxb", [P, C], BF16, kind="ExternalInput")  # u bcast
    auxf = nc.dram_tensor("auxf", [P, D], F32, kind="ExternalInput")  # knots
    tt = nc.dram_tensor("tt", [D, N], BF16, kind="ExternalInput")
    out = nc.dram_tensor("out", [N, C], BF16, kind="ExternalOutput")

    xv = xb.ap().rearrange("(p i) c -> p i c", p=P)  # [128, 32, 64]
    ov = out.ap().rearrange("(p s) c -> p s c", p=P)

    with tile.TileContext(nc) as tc:
        with (
            tc.tile_pool(name="consts", bufs=1) as consts,
            tc.tile_pool(name="sb", bufs=1) as sb,
            tc.tile_pool(name="ps_yt", bufs=1, space="PSUM") as ps_yt,
            tc.tile_pool(name="ps_small", bufs=2, space="PSUM") as ps_small,
            tc.tile_pool(name="ps_g", bufs=4, space="PSUM") as ps_g,
        ):
            x1b = sb.tile([P, NCH, C + 1], BF16)
            o_big = sb.tile([P, NCH, C], BF16)

            # two independent DMA rings (sync=Q1, gpsimd=Q0); tiny operand
            # DMAs go first on each ring so they are not stuck behind the
            # 256KB x halves, and each ring carries one x half
            HH = NCH // 2
            auxb_sb = consts.tile([P, C], BF16)
            nc.gpsimd.dma_start(out=auxb_sb[:], in_=auxb[:])
            auxf_sb = consts.tile([P, D], F32)
            nc.gpsimd.dma_start(out=auxf_sb[:], in_=auxf[:])
            wpk_sb = consts.tile([C + 1, 3 * C], BF16)
            nc.gpsimd.dma_start(out=wpk_sb[:], in_=wpk[:])
            nc.sync.dma_start(out=x1b[:, 0:HH, 0:C], in_=xv[:, 0:HH, :])
            nc.sync.dma_start(out=x1b[:, HH:NCH, 0:C], in_=xv[:, HH:NCH, :])
            tt_sb = consts.tile([D, N], BF16)
            nc.gpsimd.dma_start(out=tt_sb[:], in_=tt[:])

            oneb65 = consts.tile([C + 1, 1], BF16)
            nc.vector.memset(oneb65[:], 1.0)
            dummy = consts.tile([1, 1], F32)
            nc.vector.memset(dummy[:], 0.0)
            dummy_o = consts.tile([1, 1], F32)
            # force the Exp act-table load at t~0 instead of mid-kernel
            nc.scalar.activation(
                out=dummy_o[:], in_=dummy[:], func=mybir.ActivationFunctionType.Exp
            )
            nc.vector.memset(x1b[:, :, C : C + 1], 1.0)  # den ones column

            wv1 = wpk_sb[:, 0:C]
            wpT = wpk_sb[0:C, C : 2 * C]
            bp_row = wpk_sb[C : C + 1, 2 * C : 3 * C]

            # -------- phase A: t, L=outer(t,a), E=exp(L), ytd=[x|1]^T E
            s_cols = sb.tile([P, NCH], F32)
            xu = sb.tile([P, NCH, C], BF16)
            lmat = sb.tile([P, NCH, D], F32)
            e_all = sb.tile([P, NCH, D], BF16)
            ytd_ps = ps_yt.tile([C + 1, D], F32)

            def bcast(ap, insert_at, size):
                lst = list(ap.ap)
                lst.insert(insert_at, [0, size])
                return type(ap)(tensor=ap.tensor, offset=ap.offset, ap=lst)

            ubc_bc = bcast(auxb_sb[:], 1, QC)  # [128, (8bc), 64]
            ab_bc = bcast(auxf_sb[:], 1, QC)  # [128, (8bc), 32]

            for q in range(NQ):
                sl = slice(q * QC, (q + 1) * QC)
                # t[m] = x[m,:] . u : bf16 multiply (2x DVE), f32-accum reduce
                nc.vector.tensor_mul(xu[:, sl, :], x1b[:, sl, 0:C], ubc_bc)
                nc.vector.reduce_sum(
                    out=s_cols[:, sl],
                    in_=xu[:, sl, :],
                    axis=mybir.AxisListType.X,
                )
                # L[:, i, j] = t[:, i] * a[j]
                s_bc = bcast(s_cols[:, sl], 2, D)  # [128, 8, (32bc)]
                nc.gpsimd.tensor_mul(lmat[:, sl, :], s_bc, ab_bc)
                nc.scalar.activation(
                    out=e_all[:, sl, :],
                    in_=lmat[:, sl, :],
                    func=mybir.ActivationFunctionType.Exp,
                )
                for i in range(q * QC, (q + 1) * QC):
                    nc.tensor.matmul(
                        ytd_ps[:],
                        x1b[:, i, :],
                        e_all[:, i, :],
                        start=(i == 0),
                        stop=(i == NCH - 1),
                    )

            # -------- phase B: knot outputs
            # g = (wp-proj(wv-proj(ytd)) + den (x) bp) / den; the divide by
            # den folds into the final copy (Act scale), bp folds via the
            # den (x) bp accumulate, so no transposes and no explicit o
            ytd_sb = sb.tile([C + 1, D], BF16)
            nc.vector.tensor_copy(out=ytd_sb[:], in_=ytd_ps[:])
            denc_ps = ps_small.tile([D, 1], F32, tag="m")
            nc.tensor.matmul(
                denc_ps[:],
                ytd_sb[C : C + 1, :],
                oneb65[C : C + 1, :],
                start=True,
                stop=True,
            )
            rc_sb = sb.tile([D, 1], F32)
            nc.vector.reciprocal(out=rc_sb[:], in_=denc_ps[:])
            numT_ps = ps_small.tile([C, D], F32, tag="m")
            nc.tensor.matmul(numT_ps[:], wv1, ytd_sb[:], start=True, stop=True)
            numT_sb = sb.tile([C, D], BF16)
            nc.vector.tensor_copy(out=numT_sb[:], in_=numT_ps[:])
            g_ps = ps_small.tile([D, C], F32, tag="m")
            nc.tensor.matmul(g_ps[:], numT_sb[:], wpT, start=True, stop=False)
            nc.tensor.matmul(
                g_ps[:], ytd_sb[C : C + 1, :], bp_row, start=False, stop=True
            )
            g_sb = sb.tile([D, C], BF16)
            nc.scalar.activation(
                out=g_sb[:],
                in_=g_ps[:],
                func=mybir.ActivationFunctionType.Copy,
                scale=rc_sb[:],
            )

            # -------- phase C: expand knots to 4096 rows, natural layout
            for gidx in range(NDG):
                obp = ps_g.tile([P, DG * C], F32, tag="g")
                for k in range(DG):
                    s = gidx * DG + k
                    nc.tensor.matmul(
                        obp[:, k * C : (k + 1) * C],
                        tt_sb[:, s * P : (s + 1) * P],
                        g_sb[:],
                        start=True,
                        stop=True,
                    )
                dst = o_big[:, gidx * DG : (gidx + 1) * DG, :]
                if gidx % 2 == 0:
                    nc.vector.tensor_copy(out=dst, in_=obp[:])
                else:
                    nc.scalar.copy(out=dst, in_=obp[:])
                s0 = gidx * DG
                eng = nc.sync if gidx % 2 == 0 else nc.gpsimd
                eng.dma_start(
                    out=ov[:, s0 : s0 + DG, :],
                    in_=o_big[:, s0 : s0 + DG, :],
                )

    nc.compile()
    return nc


_nc_cache = None


def _get_nc():
    global _nc_cache
    if _nc_cache is None:
        _nc_cache = build_nc()
    return _nc_cache


def make_in_maps(x, wq, bq, wk, bk, wv, bv, wp, bp):
    f = lambda a: np.ascontiguousarray(np.asarray(a, dtype=np.float32))
    x = f(x).reshape(B, N, C)
    wq, bq, wk = f(wq), f(bq), f(wk)
    wpk = np.zeros((C + 1, 3 * C), np.float32)
    wpk[0:C, 0:C] = f(wv).T
    wpk[C, 0:C] = f(bv)
    wpk[0:C, C : 2 * C] = f(wp).T
    wpk[C, 2 * C : 3 * C] = f(bp)
    wpk = np.ascontiguousarray(wpk.astype(ml_dtypes.bfloat16))
    tt = np.ascontiguousarray(TT)
    auxf = np.ascontiguousarray(np.broadcast_to(A_KNOTS[None, :], (P, D))).astype(
        np.float32
    )
    in_maps = []
    for b in range(B):
        u = ((x[b, CENTER] @ wq.T + bq) @ wk).astype(np.float32)  # [64]
        auxb = np.ascontiguousarray(
            np.broadcast_to(u[None, :], (P, C)).astype(ml_dtypes.bfloat16)
        )
        in_maps.append(
            {
                "xb": np.ascontiguousarray(x[b].astype(ml_dtypes.bfloat16)),
                "wpk": wpk,
                "auxb": auxb,
                "auxf": auxf,
                "tt": tt,
            }
        )
    return in_maps


def kernel_with_results(trace=False, **inputs):
    in_maps = make_in_maps(**inputs)
    nc = _get_nc()
    res = run_bass_kernel_spmd(nc, in_maps, core_ids=list(range(B)), trace=trace)
    out = np.stack(
        [np.asarray(r["out"]).astype(np.float32) for r in res.results], 0
    ).reshape(B, H, W, C)
    return out, res


def kernel(**inputs):
    out, _ = kernel_with_results(**inputs)
    return out


# revision 15
# speedup vs baseline: 1.2023x; 1.1371x over previous
"""Trainium2 Bass kernel for nn_Attention_78048145703090 (sparse_attention).

Math: the reference's [N,N] attention is rank-1 structured. Every row n of the
logit matrix is a_n * t where t[m] = q_center . k_m is one shared score vector
per sample and a_n = scale * exp(1 - dist_n) depends only on the grid distance
of n from the center. Softmax rows therefore only depend on a_n, and the row
output out(a) = softmax(a*t) @ V is a smooth function of the scalar a. The
kernel evaluates D=32 uniformly spaced knots in a and expands to the 4096 rows
with a piecewise-linear interpolation matmul (interp error ~4e-5, far below
the bf16 noise floor).

Per core (one sample), m in chunks of 128 rows, pipelined in quarters:
  t    = x @ u            u = wk^T q_c folded on the host (O(C^2) prep);
                          DVE multiply (bf16, 2x mode) + reduce (f32 accum)
  L    = outer(t, a_j)    GpSimd broadcast-multiply (f32)
  E    = exp(L)           one wide Act op per quarter, bf16 out
  ytd  = [x|1]^T E        32 accumulating bf16 matmuls -> [65, 32] f32 PSUM
                          (row 64 = den, via the ones column of x1b)
  g    = proj(ytd/den)    tiny [32, 64] bf16 chain, divide via Act scale
  out  = T^T g            32 bf16 matmuls [32j,128n]^T x [32j,64c] land the
                          output in natural [n, c] layout; no transposes
x is shipped from the host already in bf16 (halves the input DMA); all PE
contractions are bf16 (fp32 matmuls cost 2 half-rate passes + double
LDWEIGHTS on TRN2), accumulation stays f32 in PSUM. Measured end-to-end
error ~4e-3 absmax-relative vs the f32 reference.

Sharding: data-parallel over B=8 across the 8 cores (one sample per core);
each core holds the full (tiny) weights.
"""

import sys

sys.path.insert(0, "/opt/trn_rl_repo")

import numpy as np

import concourse.bacc as bacc
import concourse.mybir as mybir
import concourse.tile as tile
from concourse import masks


def _install_profile_hook():
    """This image's antenv lacks axon_hooks; reconstruct it so
    run_bass_kernel_spmd(trace=True) can capture NTFF profiles. No-op for
    normal (untraced) runs."""
    import types

    try:
        import antenv.axon_hooks  # noqa: F401

        return
    except ImportError:
        pass
    try:
        import antenv

        m = types.ModuleType("antenv.axon_hooks")
        state = {"hook": None}
        m.set_axon_ntff_profile_hook = lambda h: state.__setitem__("hook", h)
        m.get_axon_ntff_profile_hook = lambda: state["hook"]
        sys.modules["antenv.axon_hooks"] = m
        antenv.axon_hooks = m
        from trn_agent_boot.trn_boot import _ntff_profile_via_ctypes

        m.set_axon_ntff_profile_hook(
            _ntff_profile_via_ctypes("/opt/axon/libaxon_pjrt.so")
        )
    except Exception:
        pass


_install_profile_hook()

from concourse.bass_utils import run_bass_kernel_spmd

B, H, W, C = 8, 64, 64, 64
N = H * W  # 4096
P = 128
NCH = N // P  # 32 chunks of 128 rows; chunk s holds rows {p*NCH+s}
CENTER = (H // 2) * W + (W // 2)  # 2080
SCALE = float(C) ** -0.5
F32 = mybir.dt.float32
BF16 = mybir.dt.bfloat16
D = 32  # interpolation knots in the temperature axis
NQ = 4  # pipeline quarters (8 chunks each)
QC = NCH // NQ  # 8
DG = 8  # output chunks per PSUM tile in the gather phase
NDG = NCH // DG  # 8

# ---- compile-time constants from the distance grid ----
import ml_dtypes

_yy, _xx = np.mgrid[0:H, 0:W]
_dist = np.sqrt(((_yy - H // 2) ** 2 + (_xx - W // 2) ** 2).astype(np.float32))
_a_n = (np.exp(np.float32(1.0) - _dist.reshape(-1)) * np.float32(SCALE)).astype(
    np.float32
)
AMAX = float(_a_n.max())
KH = AMAX / (D - 1)
A_KNOTS = (np.arange(D) * KH).astype(np.float32)  # [D]
_j = np.minimum((_a_n / KH).astype(np.int64), D - 2)
_frac = (_a_n / KH - _j).astype(np.float32)
_T = np.zeros((N, D), np.float32)
_T[np.arange(N), _j] += 1.0 - _frac
_T[np.arange(N), _j + 1] += _frac
# Tt[j, s*128 + p] = T[p*32 + s, j]: stationary strips per output chunk s
TT = np.ascontiguousarray(
    _T.reshape(P, NCH, D).transpose(2, 1, 0).reshape(D, N)
).astype(ml_dtypes.bfloat16)


def build_nc():
    nc = bacc.Bacc("TRN2", target_bir_lowering=False, debug=False, num_devices=B)
    # x pre-cast to bf16 on the host, one [128, 8, 64] quarter view per DMA
    xb = nc.dram_tensor("xb", [N, C], BF16, kind="ExternalInput")
    # wpk: [:,0:64]=[wv.T;bv], [:,64:128]=[wp.T;bp]  (bf16)
    wpk = nc.dram_tensor("wpk", [C + 1, 3 * C], BF16, kind="ExternalInput")
    auxb = nc.dram_tensor("auxb", [P, C], BF16, kind="ExternalInput")  # u bcast
    auxf = nc.dram_tensor("auxf", [P, D], F32, kind="ExternalInput")  # knots
    tt = nc.dram_tensor("tt", [D, N], BF16, kind="ExternalInput")
    out = nc.dram_tensor("out", [N, C], BF16, kind="ExternalOutput")

    xv = xb.ap().rearrange("(p i) c -> p i c", p=P)  # [128, 32, 64]
    ov = out.ap().rearrange("(p s) c -> p s c", p=P)

    with tile.TileContext(nc) as tc:
        with (
            tc.tile_pool(name="consts", bufs=1) as consts,
            tc.tile_pool(name="sb", bufs=1) as sb,
            tc.tile_pool(name="ps_yt", bufs=1, space="PSUM") as ps_yt,
            tc.tile_pool(name="ps_small", bufs=2, space="PSUM") as ps_small,
            tc.tile_pool(name="ps_g", bufs=4, space="PSUM") as ps_g,
        ):
            x1b = sb.tile([P, NCH, C + 1], BF16)
            o_big = sb.tile([P, NCH, C], BF16)

            # two independent DMA rings (sync=Q1, gpsimd=Q0); tiny operand
            # DMAs go first on each ring so they are not stuck behind the
            # 256KB x halves, and each ring carries one x half
            HH = NCH // 2
            auxb_sb = consts.tile([P, C], BF16)
            nc.gpsimd.dma_start(out=auxb_sb[:], in_=auxb[:])
            auxf_sb = consts.tile([P, D], F32)
            nc.gpsimd.dma_start(out=auxf_sb[:], in_=auxf[:])
            wpk_sb = consts.tile([C + 1, 3 * C], BF16)
            nc.gpsimd.dma_start(out=wpk_sb[:], in_=wpk[:])
            for q in range(NQ):
                nc.sync.dma_start(
                    out=x1b[:, q * QC : (q + 1) * QC, 0:C],
                    in_=xv[:, q * QC : (q + 1) * QC, :],
                )
            tt_sb = consts.tile([D, N], BF16)
            nc.gpsimd.dma_start(out=tt_sb[:], in_=tt[:])

            oneb65 = consts.tile([C + 1, 1], BF16)
            nc.vector.memset(oneb65[:], 1.0)
            dummy = consts.tile([1, 1], F32)
            nc.vector.memset(dummy[:], 0.0)
            dummy_o = consts.tile([1, 1], F32)
            # force the Exp act-table load at t~0 instead of mid-kernel
            nc.scalar.activation(
                out=dummy_o[:], in_=dummy[:], func=mybir.ActivationFunctionType.Exp
            )
            nc.vector.memset(x1b[:, :, C : C + 1], 1.0)  # den ones column

            wv1 = wpk_sb[:, 0:C]
            wpT = wpk_sb[0:C, C : 2 * C]
            bp_row = wpk_sb[C : C + 1, 2 * C : 3 * C]

            # -------- phase A: t, L=outer(t,a), E=exp(L), ytd=[x|1]^T E
            s_cols = sb.tile([P, NCH], F32)
            xu = sb.tile([P, NCH, C], BF16)
            lmat = sb.tile([P, NCH, D], F32)
            e_all = sb.tile([P, NCH, D], BF16)
            ytd_ps = ps_yt.tile([C + 1, D], F32)

            def bcast(ap, insert_at, size):
                lst = list(ap.ap)
                lst.insert(insert_at, [0, size])
                return type(ap)(tensor=ap.tensor, offset=ap.offset, ap=lst)

            ubc_bc = bcast(auxb_sb[:], 1, QC)  # [128, (8bc), 64]
            ab_bc = bcast(auxf_sb[:], 1, QC)  # [128, (8bc), 32]

            for q in range(NQ):
                sl = slice(q * QC, (q + 1) * QC)
                # t[m] = x[m,:] . u : bf16 multiply (2x DVE), f32-accum reduce
                nc.vector.tensor_mul(xu[:, sl, :], x1b[:, sl, 0:C], ubc_bc)
                nc.vector.reduce_sum(
                    out=s_cols[:, sl],
                    in_=xu[:, sl, :],
                    axis=mybir.AxisListType.X,
                )
                # L[:, i, j] = t[:, i] * a[j]
                s_bc = bcast(s_cols[:, sl], 2, D)  # [128, 8, (32bc)]
                nc.gpsimd.tensor_mul(lmat[:, sl, :], s_bc, ab_bc)
                nc.scalar.activation(
                    out=e_all[:, sl, :],
                    in_=lmat[:, sl, :],
                    func=mybir.ActivationFunctionType.Exp,
                )
                for i in range(q * QC, (q + 1) * QC):
                    nc.tensor.matmul(
                        ytd_ps[:],
                        x1b[:, i, :],
                        e_all[:, i, :],
                        start=(i == 0),
                        stop=(i == NCH - 1),
                    )

            # -------- phase B: knot outputs
            # g = (wp-proj(wv-proj(ytd)) + den (x) bp) / den; the divide by
            # den folds into the final copy (Act scale), bp folds via the
            # den (x) bp accumulate, so no transposes and no explicit o
            ytd_sb = sb.tile([C + 1, D], BF16)
            nc.vector.tensor_copy(out=ytd_sb[:], in_=ytd_ps[:])
            denc_ps = ps_small.tile([D, 1], F32, tag="m")
            nc.tensor.matmul(
                denc_ps[:],
                ytd_sb[C : C + 1, :],
                oneb65[C : C + 1, :],
                start=True,
                stop=True,
            )
            rc_sb = sb.tile([D, 1], F32)
            nc.vector.reciprocal(out=rc_sb[:], in_=denc_ps[:])
            numT_ps = ps_small.tile([C, D], F32, tag="m")
            nc.tensor.matmul(numT_ps[:], wv1, ytd_sb[:], start=True, stop=True)
            numT_sb = sb.tile([C, D], BF16)
            nc.vector.tensor_copy(out=numT_sb[:], in_=numT_ps[:])
            g_ps = ps_small.tile([D, C], F32, tag="m")
            nc.tensor.matmul(g_ps[:], numT_sb[:], wpT, start=True, stop=False)
            nc.tensor.matmul(
                g_ps[:], ytd_sb[C : C + 1, :], bp_row, start=False, stop=True
            )
            g_sb = sb.tile([D, C], BF16)
            nc.scalar.activation(
                out=g_sb[:],
                in_=g_ps[:],
                func=mybir.ActivationFunctionType.Copy,
                scale=rc_sb[:],
            )

            # -------- phase C: expand knots to 4096 rows, natural layout
            for gidx in range(NDG):
                obp = ps_g.tile([P, DG * C], F32, tag="g")
                for k in range(DG):
                    s = gidx * DG + k
                    nc.tensor.matmul(
                        obp[:, k * C : (k + 1) * C],
                        tt_sb[:, s * P : (s + 1) * P],
                        g_sb[:],
                        start=True,
                        stop=True,
                    )
                dst = o_big[:, gidx * DG : (gidx + 1) * DG, :]
                if gidx % 2 == 0:
                    nc.vector.tensor_copy(out=dst, in_=obp[:])
                else:
                    nc.scalar.copy(out=dst, in_=obp[:])
                s0 = gidx * DG
                eng = nc.sync if gidx % 2 == 0 else nc.gpsimd
                eng.dma_start(
                    out=ov[:, s0 : s0 + DG, :],
                    in_=o_big[:, s0 : s0 + DG, :],
                )

    nc.compile()
    return nc


_nc_cache = None


def _get_nc():
    global _nc_cache
    if _nc_cache is None:
        _nc_cache = build_nc()
    return _nc_cache


def make_in_maps(x, wq, bq, wk, bk, wv, bv, wp, bp):
    f = lambda a: np.ascontiguousarray(np.asarray(a, dtype=np.float32))
    x = f(x).reshape(B, N, C)
    wq, bq, wk = f(wq), f(bq), f(wk)
    wpk = np.zeros((C + 1, 3 * C), np.float32)
    wpk[0:C, 0:C] = f(wv).T
    wpk[C, 0:C] = f(bv)
    wpk[0:C, C : 2 * C] = f(wp).T
    wpk[C, 2 * C : 3 * C] = f(bp)
    wpk = np.ascontiguousarray(wpk.astype(ml_dtypes.bfloat16))
    tt = np.ascontiguousarray(TT)
    auxf = np.ascontiguousarray(np.broadcast_to(A_KNOTS[None, :], (P, D))).astype(
        np.float32
    )
    in_maps = []
    for b in range(B):
        u = ((x[b, CENTER] @ wq.T + bq) @ wk).astype(np.float32)  # [64]
        auxb = np.ascontiguousarray(
            np.broadcast_to(u[None, :], (P, C)).astype(ml_dtypes.bfloat16)
        )
        in_maps.append(
            {
                "xb": np.ascontiguousarray(x[b].astype(ml_dtypes.bfloat16)),
                "wpk": wpk,
                "auxb": auxb,
                "auxf": auxf,
                "tt": tt,
            }
        )
    return in_maps


def kernel_with_results(trace=False, **inputs):
    in_maps = make_in_maps(**inputs)
    nc = _get_nc()
    res = run_bass_kernel_spmd(nc, in_maps, core_ids=list(range(B)), trace=trace)
    out = np.stack(
        [np.asarray(r["out"]).astype(np.float32) for r in res.results], 0
    ).reshape(B, H, W, C)
    return out, res


def kernel(**inputs):
    out, _ = kernel_with_results(**inputs)
    return out


# revision 16
# speedup vs baseline: 1.2282x; 1.0216x over previous
"""Trainium2 Bass kernel for nn_Attention_78048145703090 (sparse_attention).

Math: the reference's [N,N] attention is rank-1 structured. Every row n of the
logit matrix is a_n * t where t[m] = q_center . k_m is one shared score vector
per sample and a_n = scale * exp(1 - dist_n) depends only on the grid distance
of n from the center. Softmax rows therefore only depend on a_n, and the row
output out(a) = softmax(a*t) @ V is a smooth function of the scalar a. The
kernel evaluates D=32 uniformly spaced knots in a and expands to the 4096 rows
with a piecewise-linear interpolation matmul (interp error ~4e-5, far below
the bf16 noise floor).

Per core (one sample), m in chunks of 128 rows, pipelined in quarters:
  t    = x @ u            u = wk^T q_c folded on the host (O(C^2) prep);
                          DVE multiply (bf16, 2x mode) + reduce (f32 accum)
  L    = outer(t, a_j)    GpSimd broadcast-multiply (f32)
  E    = exp(L)           one wide Act op per quarter, bf16 out
  ytd  = [x|1]^T E        32 accumulating bf16 matmuls -> [65, 32] f32 PSUM
                          (row 64 = den, via the ones column of x1b)
  g    = proj(ytd/den)    tiny [32, 64] bf16 chain, divide via Act scale
  out  = T^T g            32 bf16 matmuls [32j,128n]^T x [32j,64c] land the
                          output in natural [n, c] layout; no transposes
x is shipped from the host already in bf16 (halves the input DMA); all PE
contractions are bf16 (fp32 matmuls cost 2 half-rate passes + double
LDWEIGHTS on TRN2), accumulation stays f32 in PSUM. Measured end-to-end
error ~4e-3 absmax-relative vs the f32 reference.

Sharding: data-parallel over B=8 across the 8 cores (one sample per core);
each core holds the full (tiny) weights.
"""

import sys

sys.path.insert(0, "/opt/trn_rl_repo")

import numpy as np

import concourse.bacc as bacc
import concourse.mybir as mybir
import concourse.tile as tile
from concourse import masks


def _install_profile_hook():
    """This image's antenv lacks axon_hooks; reconstruct it so
    run_bass_kernel_spmd(trace=True) can capture NTFF profiles. No-op for
    normal (untraced) runs."""
    import types

    try:
        import antenv.axon_hooks  # noqa: F401

        return
    except ImportError:
        pass
    try:
        import antenv

        m = types.ModuleType("antenv.axon_hooks")
        state = {"hook": None}
        m.set_axon_ntff_profile_hook = lambda h: state.__setitem__("hook", h)
        m.get_axon_ntff_profile_hook = lambda: state["hook"]
        sys.modules["antenv.axon_hooks"] = m
        antenv.axon_hooks = m
        from trn_agent_boot.trn_boot import _ntff_profile_via_ctypes

        m.set_axon_ntff_profile_hook(
            _ntff_profile_via_ctypes("/opt/axon/libaxon_pjrt.so")
        )
    except Exception:
        pass


_install_profile_hook()

from concourse.bass_utils import run_bass_kernel_spmd

B, H, W, C = 8, 64, 64, 64
N = H * W  # 4096
P = 128
NCH = N // P  # 32 chunks of 128 rows; chunk s holds rows {p*NCH+s}
CENTER = (H // 2) * W + (W // 2)  # 2080
SCALE = float(C) ** -0.5
F32 = mybir.dt.float32
BF16 = mybir.dt.bfloat16
D = 32  # interpolation knots in the temperature axis
NQ = 4  # pipeline quarters (8 chunks each)
QC = NCH // NQ  # 8
DG = 8  # output chunks per PSUM tile in the gather phase
NDG = NCH // DG  # 8

# ---- compile-time constants from the distance grid ----
import ml_dtypes

_yy, _xx = np.mgrid[0:H, 0:W]
_dist = np.sqrt(((_yy - H // 2) ** 2 + (_xx - W // 2) ** 2).astype(np.float32))
_a_n = (np.exp(np.float32(1.0) - _dist.reshape(-1)) * np.float32(SCALE)).astype(
    np.float32
)
AMAX = float(_a_n.max())
KH = AMAX / (D - 1)
A_KNOTS = (np.arange(D) * KH).astype(np.float32)  # [D]
_j = np.minimum((_a_n / KH).astype(np.int64), D - 2)
_frac = (_a_n / KH - _j).astype(np.float32)
_T = np.zeros((N, D), np.float32)
_T[np.arange(N), _j] += 1.0 - _frac
_T[np.arange(N), _j + 1] += _frac
# Tt[j, s*128 + p] = T[p*32 + s, j]: stationary strips per output chunk s
TT = np.ascontiguousarray(
    _T.reshape(P, NCH, D).transpose(2, 1, 0).reshape(D, N)
).astype(ml_dtypes.bfloat16)


def build_nc():
    nc = bacc.Bacc("TRN2", target_bir_lowering=False, debug=False, num_devices=B)
    # x pre-cast to bf16 on the host, one [128, 8, 64] quarter view per DMA
    xb = nc.dram_tensor("xb", [N, C], BF16, kind="ExternalInput")
    # wpk: [:,0:64]=wv.T, [:,64:128]=wp.T  (bf16)
    wpk = nc.dram_tensor("wpk", [C, 2 * C], BF16, kind="ExternalInput")
    # auxg: [:,0:64]=u bcast, [:,64:96]=fold, row0 [96:160]=bv, [160:224]=bp
    auxb = nc.dram_tensor("auxb", [P, 3 * C + D], BF16, kind="ExternalInput")
    auxf = nc.dram_tensor("auxf", [P, D], F32, kind="ExternalInput")  # knots
    tt = nc.dram_tensor("tt", [D, N], BF16, kind="ExternalInput")
    out = nc.dram_tensor("out", [N, C], BF16, kind="ExternalOutput")

    xv = xb.ap().rearrange("(p i) c -> p i c", p=P)  # [128, 32, 64]
    ov = out.ap().rearrange("(p s) c -> p s c", p=P)

    with tile.TileContext(nc) as tc:
        with (
            tc.tile_pool(name="consts", bufs=1) as consts,
            tc.tile_pool(name="sb", bufs=1) as sb,
            tc.tile_pool(name="ps_yt", bufs=1, space="PSUM") as ps_yt,
            tc.tile_pool(name="ps_den", bufs=1, space="PSUM") as ps_den,
            tc.tile_pool(name="ps_small", bufs=2, space="PSUM") as ps_small,
            tc.tile_pool(name="ps_g", bufs=4, space="PSUM") as ps_g,
        ):
            x_sb = sb.tile([P, NCH, C], BF16)
            o_big = sb.tile([P, NCH, C], BF16)

            # two independent DMA rings (sync=Q1, gpsimd=Q0); tiny operand
            # DMAs go first on each ring so they are not stuck behind the
            # 256KB x halves, and each ring carries one x half
            HH = NCH // 2
            auxb_sb = consts.tile([P, 3 * C + D], BF16)
            nc.gpsimd.dma_start(out=auxb_sb[:], in_=auxb[:])
            auxf_sb = consts.tile([P, D], F32)
            nc.gpsimd.dma_start(out=auxf_sb[:], in_=auxf[:])
            wpk_sb = consts.tile([C, 2 * C], BF16)
            nc.gpsimd.dma_start(out=wpk_sb[:], in_=wpk[:])
            for q in range(NQ):
                nc.sync.dma_start(
                    out=x_sb[:, q * QC : (q + 1) * QC, :],
                    in_=xv[:, q * QC : (q + 1) * QC, :],
                )
            tt_sb = consts.tile([D, N], BF16)
            nc.gpsimd.dma_start(out=tt_sb[:], in_=tt[:])

            ones_col = consts.tile([P, 1], BF16)
            nc.vector.memset(ones_col[:], 1.0)
            dummy = consts.tile([1, 1], F32)
            nc.vector.memset(dummy[:], 0.0)
            dummy_o = consts.tile([1, 1], F32)
            # force the Exp act-table load at t~0 instead of mid-kernel
            nc.scalar.activation(
                out=dummy_o[:], in_=dummy[:], func=mybir.ActivationFunctionType.Exp
            )
            ubc = auxb_sb[:, 0:C]
            fold = auxb_sb[:, C : C + D]
            bv_row = auxb_sb[0:1, C + D : 2 * C + D]
            bp_row = auxb_sb[0:1, 2 * C + D : 3 * C + D]
            wvT = wpk_sb[:, 0:C]
            wpT = wpk_sb[:, C : 2 * C]

            # -------- phase A: t, L=outer(t,a), E=exp(L), ytd=[x|1]^T E
            s_cols = sb.tile([P, NCH], F32)
            xu = sb.tile([P, NCH, C], BF16)
            lmat = sb.tile([P, NCH, D], F32)
            e_all = sb.tile([P, NCH, D], BF16)
            yt_ps = ps_yt.tile([C, D], F32)
            deng_ps = ps_den.tile([P, 1], F32)

            def bcast(ap, insert_at, size):
                lst = list(ap.ap)
                lst.insert(insert_at, [0, size])
                return type(ap)(tensor=ap.tensor, offset=ap.offset, ap=lst)

            ubc_bc = bcast(ubc, 1, QC)  # [128, (8bc), 64]
            ab_bc = bcast(auxf_sb[:], 1, QC)  # [128, (8bc), 32]

            for q in range(NQ):
                sl = slice(q * QC, (q + 1) * QC)
                # t[m] = x[m,:] . u : bf16 multiply (2x DVE), f32-accum reduce
                nc.vector.tensor_mul(xu[:, sl, :], x_sb[:, sl, :], ubc_bc)
                nc.vector.reduce_sum(
                    out=s_cols[:, sl],
                    in_=xu[:, sl, :],
                    axis=mybir.AxisListType.X,
                )
                # L[:, i, j] = t[:, i] * a[j]
                s_bc = bcast(s_cols[:, sl], 2, D)  # [128, 8, (32bc)]
                nc.gpsimd.tensor_mul(lmat[:, sl, :], s_bc, ab_bc)
                nc.scalar.activation(
                    out=e_all[:, sl, :],
                    in_=lmat[:, sl, :],
                    func=mybir.ActivationFunctionType.Exp,
                )
                for i in range(q * QC, (q + 1) * QC):
                    nc.tensor.matmul(
                        yt_ps[:],
                        x_sb[:, i, :],
                        e_all[:, i, :],
                        start=(i == 0),
                        stop=(i == NCH - 1),
                    )
                # den partials: ones^T E over 4-chunk groups of e as stationary
                for gi in range(q * 2, q * 2 + 2):
                    nc.tensor.matmul(
                        deng_ps[:],
                        e_all[:, gi * 4 : (gi + 1) * 4, :],
                        ones_col[:],
                        start=(gi == 0),
                        stop=(gi == 7),
                    )

            # -------- phase B: knot outputs
            # g = (wp-proj(wv-proj(yt) + bv (x) den) + den (x) bp) / den;
            # the divide folds into the final copy (Act scale), bv/bp fold
            # via den outer-product accumulates, so no transposes needed
            deng_sb = sb.tile([P, 1], BF16)
            nc.vector.tensor_copy(out=deng_sb[:], in_=deng_ps[:])
            yt_sb = sb.tile([C, D], BF16)
            nc.vector.tensor_copy(out=yt_sb[:], in_=yt_ps[:])
            denc_ps = ps_small.tile([D, 1], F32, tag="m")
            nc.tensor.matmul(denc_ps[:], fold, deng_sb[:], start=True, stop=True)
            denr_ps = ps_small.tile([1, D], F32, tag="m")
            nc.tensor.matmul(denr_ps[:], deng_sb[:], fold, start=True, stop=True)
            rc_sb = sb.tile([D, 1], F32)
            nc.vector.reciprocal(out=rc_sb[:], in_=denc_ps[:])
            denr_sb = sb.tile([1, D], BF16)
            nc.scalar.copy(out=denr_sb[:], in_=denr_ps[:])
            numT_ps = ps_small.tile([C, D], F32, tag="m")
            nc.tensor.matmul(numT_ps[:], wvT, yt_sb[:], start=True, stop=False)
            nc.tensor.matmul(numT_ps[:], bv_row, denr_sb[:], start=False, stop=True)
            numT_sb = sb.tile([C, D], BF16)
            nc.vector.tensor_copy(out=numT_sb[:], in_=numT_ps[:])
            g_ps = ps_small.tile([D, C], F32, tag="m")
            nc.tensor.matmul(g_ps[:], numT_sb[:], wpT, start=True, stop=False)
            nc.tensor.matmul(g_ps[:], denr_sb[:], bp_row, start=False, stop=True)
            g_sb = sb.tile([D, C], BF16)
            nc.scalar.activation(
                out=g_sb[:],
                in_=g_ps[:],
                func=mybir.ActivationFunctionType.Copy,
                scale=rc_sb[:],
            )

            # -------- phase C: expand knots to 4096 rows, natural layout
            for gidx in range(NDG):
                obp = ps_g.tile([P, DG * C], F32, tag="g")
                for k in range(DG):
                    s = gidx * DG + k
                    nc.tensor.matmul(
                        obp[:, k * C : (k + 1) * C],
                        tt_sb[:, s * P : (s + 1) * P],
                        g_sb[:],
                        start=True,
                        stop=True,
                    )
                dst = o_big[:, gidx * DG : (gidx + 1) * DG, :]
                if gidx % 2 == 0:
                    nc.vector.tensor_copy(out=dst, in_=obp[:])
                else:
                    nc.scalar.copy(out=dst, in_=obp[:])
                s0 = gidx * DG
                eng = nc.sync if gidx % 2 == 0 else nc.gpsimd
                eng.dma_start(
                    out=ov[:, s0 : s0 + DG, :],
                    in_=o_big[:, s0 : s0 + DG, :],
                )

    nc.compile()
    return nc


_nc_cache = None


def _get_nc():
    global _nc_cache
    if _nc_cache is None:
        _nc_cache = build_nc()
    return _nc_cache


def make_in_maps(x, wq, bq, wk, bk, wv, bv, wp, bp):
    f = lambda a: np.ascontiguousarray(np.asarray(a, dtype=np.float32))
    x = f(x).reshape(B, N, C)
    wq, bq, wk = f(wq), f(bq), f(wk)
    wpk = np.zeros((C, 2 * C), np.float32)
    wpk[:, 0:C] = f(wv).T
    wpk[:, C : 2 * C] = f(wp).T
    wpk = np.ascontiguousarray(wpk.astype(ml_dtypes.bfloat16))
    fold = (np.arange(P)[:, None] % D == np.arange(D)[None, :]).astype(np.float32)
    tt = np.ascontiguousarray(TT)
    auxf = np.ascontiguousarray(np.broadcast_to(A_KNOTS[None, :], (P, D))).astype(
        np.float32
    )
    in_maps = []
    for b in range(B):
        u = ((x[b, CENTER] @ wq.T + bq) @ wk).astype(np.float32)  # [64]
        auxg = np.zeros((P, 3 * C + D), np.float32)
        auxg[:, 0:C] = u[None, :]
        auxg[:, C : C + D] = fold
        auxg[0, C + D : 2 * C + D] = f(bv)
        auxg[0, 2 * C + D : 3 * C + D] = f(bp)
        auxb = np.ascontiguousarray(auxg.astype(ml_dtypes.bfloat16))
        in_maps.append(
            {
                "xb": np.ascontiguousarray(x[b].astype(ml_dtypes.bfloat16)),
                "wpk": wpk,
                "auxb": auxb,
                "auxf": auxf,
                "tt": tt,
            }
        )
    return in_maps


def kernel_with_results(trace=False, **inputs):
    in_maps = make_in_maps(**inputs)
    nc = _get_nc()
    res = run_bass_kernel_spmd(nc, in_maps, core_ids=list(range(B)), trace=trace)
    out = np.stack(
        [np.asarray(r["out"]).astype(np.float32) for r in res.results], 0
    ).reshape(B, H, W, C)
    return out, res


def kernel(**inputs):
    out, _ = kernel_with_results(**inputs)
    return out
